# revision 18
# baseline (speedup 1.0000x reference)
"""RBF kernel feature map: out[b, r] = exp(-||x[b] - refs[r]||^2).

Computed via the GEMM expansion on 8 NeuronCores, data-parallel over the
batch dim of x (2048 rows per core), refs replicated.

Per-core device kernel, per [128, 2048] output tile:
    psum[b, r] = sum_d x[b,d]*refs[r,d] - 0.5*r_sq[r] - 0.5*x_sq[b]
      (4 matmuls of N=512 side by side into one 4-bank PSUM tile)
    out[b, r]  = exp(2 * psum[b, r])   (ONE ScalarE Exp over N=2048)

The contraction packs all the norm terms into 4 extra K rows (vs ones-rows
on the opposite side). Each norm term is split hi (bf16-representable) +
lo (remainder) so the full-rate fp32r matmul path keeps the ~30..300
magnitude norm terms accurate even if fp32r quantizes inputs to ~tf32
internally.

Perf notes vs the v1 kernel (79.6us -> ~54.5us measured):
  - input DRAM param padded from [68, B+R] to [128, B+R]: a 68-partition
    DMA was served by only 4 SDMA engines (77 GB/s, 14.4us startup
    stall); 128 partitions spread over all 16 engines (~330 GB/s). The
    60 zero rows cost bytes but not time (engine count ~ partitions/8).
  - output stored as bf16 and upcast on the host: halves the 16 MiB/core
    store traffic that ran at the ~345 GB/s per-core HBM ceiling.
    exp() outputs are in [0, ~1e-13]; bf16 adds <=2e-3 relative error
    against a 2e-2 budget (measured baseline error 2.8e-3).
  - one Exp ACTIVATE per [128, 2048] PSUM region instead of four per
    [128, 512]: the ~170-350 cycle per-instruction bubble made 64 small
    ACTs cost 59.7us of ScalarE time; 16 big ones cost ~31.4us.
  - input loaded in need-ordered chunks: x cols 0:256 on the scalar
    HWDGE ring, refs as four rc-sized [128, 512] chunks first on the
    sync ring (rest of x behind them). The PE stream - the steady-state
    pacer - starts after just x0+r0 (~384 KB) instead of the full 2 MiB,
    and each bt=0 matmul's refs chunk lands just ahead of its use.
  - middle tiles store in 1 MiB pairs (DRAM AP rearranged "(b p) r ->
    p b r") to cut sync-queue instructions and dependency events; the
    last tile ends with a half-size store issued from the scalar queue.
  - steady state is PE-paced at ~520ns per 512-col fp32r matmul (the PE
    stays at 1.2 GHz here - HAM warm-up via dummy matmuls was tried and
    does not engage; measured identical 628ns for bf16 and fp32r), with
    the 1.96us Exp per tile just underneath. ~7us NEFF prologue and
    ~3us event-drain epilogue are runtime-fixed.

Uses bacc.Bacc (not raw bass.Bass): TRN2 instructions carry at most one
semaphore wait, and Bacc.compile()'s generate_event_semaphores pass
legalizes the multi-wait instructions Tile emits.
"""

import numpy as np

N_CORES = 8
B, D, R = 16384, 64, 2048
B_SHARD = B // N_CORES  # 2048
K = D + 4  # 64 data rows + r_sq hi/lo + x_sq hi/lo rows
KP = 128  # padded partition count so the input DMA uses all 16 SDMA engines
BT = 128  # batch rows per tile (PSUM partition dim)
RC = 512  # refs cols per matmul (max fp32 moving free dim)

MM_DT = "float32r"  # full-rate fp32 matmul mode; "float32" = 4x slower, exact


def _build_nc():
    from contextlib import ExitStack

    import concourse.tile as tile
    from concourse import bacc, mybir

    mm_dt = getattr(mybir.dt, MM_DT)
    bf16 = mybir.dt.bfloat16

    nc = bacc.Bacc(None)
    # x-shard transpose and refs transpose concatenated along the free dim so
    # ONE DMA loads both matmul operands.
    inT_aug = nc.declare_dram_parameter(
        "inT_aug", [KP, B_SHARD + R], mm_dt, isOutput=False
    )
    out = nc.declare_dram_parameter("out", [B_SHARD, R], bf16, isOutput=True)

    n_bt = B_SHARD // BT
    n_rc = R // RC

    with tile.TileContext(nc) as tc, ExitStack() as ctx:
        consts = ctx.enter_context(tc.tile_pool(name="consts", bufs=1))
        outs = ctx.enter_context(tc.tile_pool(name="outs", bufs=4))
        psums = ctx.enter_context(tc.tile_pool(name="psums", bufs=2, space="PSUM"))

        # Input load, chunked for load/compute overlap and ordered by first
        # need, so the bt=0 matmuls can start ~4us before the full 2 MiB lands.
        x0 = consts.tile([KP, 256], mm_dt)
        r0 = consts.tile([KP, 512], mm_dt)
        r1 = consts.tile([KP, 512], mm_dt)
        r2 = consts.tile([KP, 512], mm_dt)
        r3 = consts.tile([KP, 512], mm_dt)
        x1 = consts.tile([KP, 768], mm_dt)
        x2 = consts.tile([KP, 1024], mm_dt)
        # The PE stream (the steady-state pacer) starts as soon as x0 plus the
        # FIRST 512-col refs chunk land, and the remaining refs chunks arrive
        # just ahead of the matmul that needs them: refs stream in rc-sized
        # [128, 512] chunks on the sync HWDGE ring while the tiny x0 rides the
        # scalar HWDGE ring concurrently. The rest of x queues BEHIND refs on
        # sync so it cannot steal bandwidth from the critical path.
        nc.scalar.dma_start(out=x0, in_=inT_aug[:, 0:256])
        refs_sb = [r0, r1, r2, r3]
        for i, rt in enumerate(refs_sb):
            nc.sync.dma_start(
                out=rt, in_=inT_aug[:, B_SHARD + i * RC : B_SHARD + (i + 1) * RC]
            )
        nc.sync.dma_start(out=x1, in_=inT_aug[:, 256:1024])
        nc.sync.dma_start(out=x2, in_=inT_aug[:, 1024:2048])
        refs_off = [0, 0, 0, 0]

        def x_slice(bt):
            if bt < 2:
                return x0, bt * BT
            if bt < 8:
                return x1, (bt - 2) * BT
            return x2, (bt - 8) * BT

        h = R // 2

        # --- tiles 0..14: paired stores (one 1 MiB DMA per two tiles) to cut
        # sync-queue instruction count and dependency-event count (the event
        # drain at kernel end costs ~90ns per event across all engines);
        # bt14 stores alone.
        # Store groups: quads for the bulk, a triple at the end; fewer
        # stores = fewer sync-queue instructions and dependency events.
        GROUPS = [(0, 4), (4, 4), (8, 4), (12, 3)]
        GROUP_OF = {}
        for gi, (g0, gn) in enumerate(GROUPS):
            for b in range(g0, g0 + gn):
                GROUP_OF[b] = gi
        grp_sb = None
        for bt in range(0, n_bt - 1):
            x_sb, x_off = x_slice(bt)
            ps = psums.tile([BT, R], mybir.dt.float32, tag="ps")
            for rc in range(n_rc):
                nc.tensor.matmul(
                    ps[:, rc * RC : (rc + 1) * RC],
                    lhsT=x_sb[0:K, x_off : x_off + BT],
                    rhs=refs_sb[rc][0:K, refs_off[rc] : refs_off[rc] + RC],
                    start=True,
                    stop=True,
                )
            gi = GROUP_OF[bt]
            g0, gn = GROUPS[gi]
            if bt == g0:
                grp_sb = outs.tile([BT, gn * R], bf16, tag="grp_sb")
            off = bt - g0
            nc.scalar.activation(
                grp_sb[:, off * R : (off + 1) * R],
                ps,
                mybir.ActivationFunctionType.Exp,
                bias=0.0,
                scale=2.0,
            )
            if bt == g0 + gn - 1:
                # One store per group: DRAM side rearranged so partition p's
                # line maps to rows {g0..g0+gn-1}*BT + p.
                nc.sync.dma_start(
                    out=out[g0 * BT : (g0 + gn) * BT, :].rearrange(
                        "(b p) r -> p b r", b=gn
                    ),
                    in_=grp_sb,
                )

        # --- last tile: serial tail ends with a half-size store issued from
        # the scalar queue itself (same-engine program order, no sem hop).
        bt = n_bt - 1
        x_sb, x_off = x_slice(bt)
        ps = psums.tile([BT, R], mybir.dt.float32, tag="ps")
        for rc in range(n_rc):
            nc.tensor.matmul(
                ps[:, rc * RC : (rc + 1) * RC],
                lhsT=x_sb[0:K, x_off : x_off + BT],
                rhs=refs_sb[rc][0:K, refs_off[rc] : refs_off[rc] + RC],
                start=True,
                stop=True,
            )
        out_sb = outs.tile([BT, R], bf16, tag="out_sb")
        q = 3 * R // 4
        for lo, hi, eng in ((0, q, nc.sync), (q, R, nc.scalar)):
            nc.scalar.activation(
                out_sb[:, lo:hi],
                ps[:, lo:hi],
                mybir.ActivationFunctionType.Exp,
                bias=0.0,
                scale=2.0,
            )
            eng.dma_start(
                out=out[bt * BT : (bt + 1) * BT, lo:hi],
                in_=out_sb[:, lo:hi],
            )

    nc.compile()
    return nc


def _hi_lo(v):
    """Split fp64 vector into bf16-representable hi + fp32 remainder lo."""
    import ml_dtypes

    hi = v.astype(np.float32).astype(ml_dtypes.bfloat16).astype(np.float32)
    lo = (v - hi).astype(np.float32)
    return hi, lo


def make_in_maps(x, refs):
    """Host-side prep: shard/transpose x, pack norm terms as extra K rows."""
    x = np.ascontiguousarray(x, dtype=np.float32)
    refs = np.ascontiguousarray(refs, dtype=np.float32)

    r_hi, r_lo = _hi_lo(0.5 * (refs.astype(np.float64) ** 2).sum(axis=1))
    x_sq = 0.5 * (x.astype(np.float64) ** 2).sum(axis=1)  # [B]

    in_maps = []
    for c in range(N_CORES):
        sl = slice(c * B_SHARD, (c + 1) * B_SHARD)
        x_hi, x_lo = _hi_lo(x_sq[sl])
        inT_aug = np.zeros((KP, B_SHARD + R), np.float32)
        inT_aug[:D, :B_SHARD] = x[sl].T
        inT_aug[D, :B_SHARD] = 1.0
        inT_aug[D + 1, :B_SHARD] = 1.0
        inT_aug[D + 2, :B_SHARD] = -x_hi
        inT_aug[D + 3, :B_SHARD] = -x_lo
        inT_aug[:D, B_SHARD:] = refs.T
        inT_aug[D, B_SHARD:] = -r_hi
        inT_aug[D + 1, B_SHARD:] = -r_lo
        inT_aug[D + 2, B_SHARD:] = 1.0
        inT_aug[D + 3, B_SHARD:] = 1.0
        in_maps.append({"inT_aug": inT_aug})
    return in_maps


_NC_CACHE = None


def get_nc():
    global _NC_CACHE
    if _NC_CACHE is None:
        _NC_CACHE = _build_nc()
    return _NC_CACHE


def kernel(x, refs):
    from concourse.bass_utils import run_bass_kernel_spmd

    in_maps = make_in_maps(x, refs)
    res = run_bass_kernel_spmd(
        get_nc(), in_maps, core_ids=list(range(N_CORES))
    ).results
    return np.concatenate(
        [res[c]["out"].astype(np.float32) for c in range(N_CORES)], axis=0
    )


# revision 19
# speedup vs baseline: 1.0601x; 1.0601x over previous
"""RBF kernel feature map: out[b, r] = exp(-||x[b] - refs[r]||^2).

Computed via the GEMM expansion on 8 NeuronCores, data-parallel over the
batch dim of x (2048 rows per core), refs replicated.

Per-core device kernel, per [128, 2048] output tile:
    psum[b, r] = sum_d x[b,d]*refs[r,d] - 0.5*r_sq[r] - 0.5*x_sq[b]
      (4 matmuls of N=512 side by side into one 4-bank PSUM tile)
    out[b, r]  = exp(2 * psum[b, r])   (ONE ScalarE Exp over N=2048)

The contraction packs all the norm terms into 4 extra K rows (vs ones-rows
on the opposite side). Each norm term is split hi (bf16-representable) +
lo (remainder) so the full-rate fp32r matmul path keeps the ~30..300
magnitude norm terms accurate even if fp32r quantizes inputs to ~tf32
internally.

Perf notes vs the v1 kernel (79.6us -> ~54.5us measured):
  - input DRAM param padded from [68, B+R] to [128, B+R]: a 68-partition
    DMA was served by only 4 SDMA engines (77 GB/s, 14.4us startup
    stall); 128 partitions spread over all 16 engines (~330 GB/s). The
    60 zero rows cost bytes but not time (engine count ~ partitions/8).
  - output stored as bf16 and upcast on the host: halves the 16 MiB/core
    store traffic that ran at the ~345 GB/s per-core HBM ceiling.
    exp() outputs are in [0, ~1e-13]; bf16 adds <=2e-3 relative error
    against a 2e-2 budget (measured baseline error 2.8e-3).
  - one Exp ACTIVATE per [128, 2048] PSUM region instead of four per
    [128, 512]: the ~170-350 cycle per-instruction bubble made 64 small
    ACTs cost 59.7us of ScalarE time; 16 big ones cost ~31.4us.
  - input loaded in need-ordered chunks: x cols 0:256 on the scalar
    HWDGE ring, refs as four rc-sized [128, 512] chunks first on the
    sync ring (rest of x behind them). The PE stream - the steady-state
    pacer - starts after just x0+r0 (~384 KB) instead of the full 2 MiB,
    and each bt=0 matmul's refs chunk lands just ahead of its use.
  - middle tiles store in 1 MiB pairs (DRAM AP rearranged "(b p) r ->
    p b r") to cut sync-queue instructions and dependency events; the
    last tile ends with a half-size store issued from the scalar queue.
  - steady state is PE-paced at ~520ns per 512-col fp32r matmul (the PE
    stays at 1.2 GHz here - HAM warm-up via dummy matmuls was tried and
    does not engage; measured identical 628ns for bf16 and fp32r), with
    the 1.96us Exp per tile just underneath. ~7us NEFF prologue and
    ~3us event-drain epilogue are runtime-fixed.

Uses bacc.Bacc (not raw bass.Bass): TRN2 instructions carry at most one
semaphore wait, and Bacc.compile()'s generate_event_semaphores pass
legalizes the multi-wait instructions Tile emits.
"""

import numpy as np

N_CORES = 8
B, D, R = 16384, 64, 2048
B_SHARD = B // N_CORES  # 2048
K = D + 4  # 64 data rows + r_sq hi/lo + x_sq hi/lo rows
KP = 128  # padded partition count so the input DMA uses all 16 SDMA engines
BT = 128  # batch rows per tile (PSUM partition dim)
RC = 512  # refs cols per matmul (max fp32 moving free dim)

MM_DT = "float32r"  # full-rate fp32 matmul mode; "float32" = 4x slower, exact


def _build_nc():
    from contextlib import ExitStack

    import concourse.tile as tile
    from concourse import bacc, mybir

    mm_dt = getattr(mybir.dt, MM_DT)
    bf16 = mybir.dt.bfloat16

    nc = bacc.Bacc(None)
    # x-shard transpose and refs transpose concatenated along the free dim so
    # ONE DMA loads both matmul operands.
    inT_aug = nc.declare_dram_parameter(
        "inT_aug", [KP, B_SHARD + R], mm_dt, isOutput=False
    )
    out = nc.declare_dram_parameter("out", [B_SHARD, R], bf16, isOutput=True)

    n_bt = B_SHARD // BT
    n_rc = R // RC

    with tile.TileContext(nc) as tc, ExitStack() as ctx:
        consts = ctx.enter_context(tc.tile_pool(name="consts", bufs=1))
        outs = ctx.enter_context(tc.tile_pool(name="outs", bufs=4))
        psums = ctx.enter_context(tc.tile_pool(name="psums", bufs=2, space="PSUM"))

        # Input load, chunked for load/compute overlap and ordered by first
        # need, so the bt=0 matmuls can start ~4us before the full 2 MiB lands.
        x0 = consts.tile([KP, 256], mm_dt)
        r0 = consts.tile([KP, 512], mm_dt)
        r1 = consts.tile([KP, 512], mm_dt)
        r2 = consts.tile([KP, 512], mm_dt)
        r3 = consts.tile([KP, 512], mm_dt)
        x1 = consts.tile([KP, 768], mm_dt)
        x2 = consts.tile([KP, 1024], mm_dt)
        # The PE stream (the steady-state pacer) starts as soon as x0 plus the
        # FIRST 512-col refs chunk land, and the remaining refs chunks arrive
        # just ahead of the matmul that needs them: refs stream in rc-sized
        # [128, 512] chunks on the sync HWDGE ring while the tiny x0 rides the
        # scalar HWDGE ring concurrently. The rest of x queues BEHIND refs on
        # sync so it cannot steal bandwidth from the critical path.
        nc.scalar.dma_start(out=x0, in_=inT_aug[:, 0:256])
        refs_sb = [r0, r1, r2, r3]
        for i, rt in enumerate(refs_sb):
            nc.sync.dma_start(
                out=rt, in_=inT_aug[:, B_SHARD + i * RC : B_SHARD + (i + 1) * RC]
            )
        nc.sync.dma_start(out=x1, in_=inT_aug[:, 256:1024])
        nc.sync.dma_start(out=x2, in_=inT_aug[:, 1024:2048])
        refs_off = [0, 0, 0, 0]

        def x_slice(bt):
            if bt < 2:
                return x0, bt * BT
            if bt < 8:
                return x1, (bt - 2) * BT
            return x2, (bt - 8) * BT

        h = R // 2

        # --- tiles 0..14: paired stores (one 1 MiB DMA per two tiles) to cut
        # sync-queue instruction count and dependency-event count (the event
        # drain at kernel end costs ~90ns per event across all engines);
        # bt14 stores alone.
        pair_sb = None
        for bt in range(0, n_bt - 1):
            x_sb, x_off = x_slice(bt)
            ps = psums.tile([BT, R], mybir.dt.float32, tag="ps")
            for rc in range(n_rc):
                nc.tensor.matmul(
                    ps[:, rc * RC : (rc + 1) * RC],
                    lhsT=x_sb[0:K, x_off : x_off + BT],
                    rhs=refs_sb[rc][0:K, refs_off[rc] : refs_off[rc] + RC],
                    start=True,
                    stop=True,
                )
            if bt == n_bt - 2:
                sngl_sb = outs.tile([BT, R], bf16, tag="out_sb")
                nc.scalar.activation(
                    sngl_sb,
                    ps,
                    mybir.ActivationFunctionType.Exp,
                    bias=0.0,
                    scale=2.0,
                )
                nc.sync.dma_start(
                    out=out[bt * BT : (bt + 1) * BT, :], in_=sngl_sb
                )
                continue
            if bt % 2 == 0:
                pair_sb = outs.tile([BT, 2 * R], bf16, tag="pair_sb")
            half = bt % 2
            nc.scalar.activation(
                pair_sb[:, half * R : (half + 1) * R],
                ps,
                mybir.ActivationFunctionType.Exp,
                bias=0.0,
                scale=2.0,
            )
            if bt % 2 == 1:
                # One 1 MiB store for the two tiles: DRAM side rearranged so
                # partition p's line maps to rows {bt-1, bt}*BT + p.
                nc.sync.dma_start(
                    out=out[(bt - 1) * BT : (bt + 1) * BT, :].rearrange(
                        "(b p) r -> p b r", b=2
                    ),
                    in_=pair_sb,
                )

        # --- last tile: serial tail ends with a half-size store issued from
        # the scalar queue itself (same-engine program order, no sem hop).
        bt = n_bt - 1
        x_sb, x_off = x_slice(bt)
        ps = psums.tile([BT, R], mybir.dt.float32, tag="ps")
        for rc in range(n_rc):
            nc.tensor.matmul(
                ps[:, rc * RC : (rc + 1) * RC],
                lhsT=x_sb[0:K, x_off : x_off + BT],
                rhs=refs_sb[rc][0:K, refs_off[rc] : refs_off[rc] + RC],
                start=True,
                stop=True,
            )
        out_sb = outs.tile([BT, R], bf16, tag="out_sb")
        for half in range(2):
            nc.scalar.activation(
                out_sb[:, half * h : (half + 1) * h],
                ps[:, half * h : (half + 1) * h],
                mybir.ActivationFunctionType.Exp,
                bias=0.0,
                scale=2.0,
            )
            dma_eng = nc.scalar if half == 1 else nc.sync
            dma_eng.dma_start(
                out=out[bt * BT : (bt + 1) * BT, half * h : (half + 1) * h],
                in_=out_sb[:, half * h : (half + 1) * h],
            )

    nc.compile()
    return nc


def _hi_lo(v):
    """Split fp64 vector into bf16-representable hi + fp32 remainder lo."""
    import ml_dtypes

    hi = v.astype(np.float32).astype(ml_dtypes.bfloat16).astype(np.float32)
    lo = (v - hi).astype(np.float32)
    return hi, lo


def make_in_maps(x, refs):
    """Host-side prep: shard/transpose x, pack norm terms as extra K rows."""
    x = np.ascontiguousarray(x, dtype=np.float32)
    refs = np.ascontiguousarray(refs, dtype=np.float32)

    r_hi, r_lo = _hi_lo(0.5 * (refs.astype(np.float64) ** 2).sum(axis=1))
    x_sq = 0.5 * (x.astype(np.float64) ** 2).sum(axis=1)  # [B]

    in_maps = []
    for c in range(N_CORES):
        sl = slice(c * B_SHARD, (c + 1) * B_SHARD)
        x_hi, x_lo = _hi_lo(x_sq[sl])
        inT_aug = np.zeros((KP, B_SHARD + R), np.float32)
        inT_aug[:D, :B_SHARD] = x[sl].T
        inT_aug[D, :B_SHARD] = 1.0
        inT_aug[D + 1, :B_SHARD] = 1.0
        inT_aug[D + 2, :B_SHARD] = -x_hi
        inT_aug[D + 3, :B_SHARD] = -x_lo
        inT_aug[:D, B_SHARD:] = refs.T
        inT_aug[D, B_SHARD:] = -r_hi
        inT_aug[D + 1, B_SHARD:] = -r_lo
        inT_aug[D + 2, B_SHARD:] = 1.0
        inT_aug[D + 3, B_SHARD:] = 1.0
        in_maps.append({"inT_aug": inT_aug})
    return in_maps


_NC_CACHE = None


def get_nc():
    global _NC_CACHE
    if _NC_CACHE is None:
        _NC_CACHE = _build_nc()
    return _NC_CACHE


def kernel(x, refs):
    from concourse.bass_utils import run_bass_kernel_spmd

    in_maps = make_in_maps(x, refs)
    res = run_bass_kernel_spmd(
        get_nc(), in_maps, core_ids=list(range(N_CORES))
    ).results
    return np.concatenate(
        [res[c]["out"].astype(np.float32) for c in range(N_CORES)], axis=0
    )


# revision 20
# speedup vs baseline: 1.0838x; 1.0224x over previous
"""RBF kernel feature map: out[b, r] = exp(-||x[b] - refs[r]||^2).

Computed via the GEMM expansion on 8 NeuronCores, data-parallel over the
batch dim of x (2048 rows per core), refs replicated.

Per-core device kernel, per [128, 2048] output tile:
    psum[b, r] = sum_d x[b,d]*refs[r,d] - 0.5*r_sq[r] - 0.5*x_sq[b]
      (4 matmuls of N=512 side by side into one 4-bank PSUM tile)
    out[b, r]  = exp(2 * psum[b, r])   (ONE ScalarE Exp over N=2048)

The contraction packs all the norm terms into 4 extra K rows (vs ones-rows
on the opposite side). Each norm term is split hi (bf16-representable) +
lo (remainder) so the full-rate fp32r matmul path keeps the ~30..300
magnitude norm terms accurate even if fp32r quantizes inputs to ~tf32
internally.

Perf notes vs the v1 kernel (79.6us -> ~54.5us measured):
  - input DRAM param padded from [68, B+R] to [128, B+R]: a 68-partition
    DMA was served by only 4 SDMA engines (77 GB/s, 14.4us startup
    stall); 128 partitions spread over all 16 engines (~330 GB/s). The
    60 zero rows cost bytes but not time (engine count ~ partitions/8).
  - output stored as bf16 and upcast on the host: halves the 16 MiB/core
    store traffic that ran at the ~345 GB/s per-core HBM ceiling.
    exp() outputs are in [0, ~1e-13]; bf16 adds <=2e-3 relative error
    against a 2e-2 budget (measured baseline error 2.8e-3).
  - one Exp ACTIVATE per [128, 2048] PSUM region instead of four per
    [128, 512]: the ~170-350 cycle per-instruction bubble made 64 small
    ACTs cost 59.7us of ScalarE time; 16 big ones cost ~31.4us.
  - input loaded in need-ordered chunks: x cols 0:256 on the scalar
    HWDGE ring, refs as four rc-sized [128, 512] chunks first on the
    sync ring (rest of x behind them). The PE stream - the steady-state
    pacer - starts after just x0+r0 (~384 KB) instead of the full 2 MiB,
    and each bt=0 matmul's refs chunk lands just ahead of its use.
  - middle tiles store in 1 MiB pairs (DRAM AP rearranged "(b p) r ->
    p b r") to cut sync-queue instructions and dependency events; the
    last tile ends with a half-size store issued from the scalar queue.
  - steady state is PE-paced at ~520ns per 512-col fp32r matmul (the PE
    stays at 1.2 GHz here - HAM warm-up via dummy matmuls was tried and
    does not engage; measured identical 628ns for bf16 and fp32r), with
    the 1.96us Exp per tile just underneath. ~7us NEFF prologue and
    ~3us event-drain epilogue are runtime-fixed.

Uses bacc.Bacc (not raw bass.Bass): TRN2 instructions carry at most one
semaphore wait, and Bacc.compile()'s generate_event_semaphores pass
legalizes the multi-wait instructions Tile emits.
"""

import numpy as np

N_CORES = 8
B, D, R = 16384, 64, 2048
B_SHARD = B // N_CORES  # 2048
K = D + 4  # 64 data rows + r_sq hi/lo + x_sq hi/lo rows
KP = 128  # padded partition count so the input DMA uses all 16 SDMA engines
BT = 128  # batch rows per tile (PSUM partition dim)
RC = 512  # refs cols per matmul (max fp32 moving free dim)

MM_DT = "float32r"  # full-rate fp32 matmul mode; "float32" = 4x slower, exact


def _build_nc():
    from contextlib import ExitStack

    import concourse.tile as tile
    from concourse import bacc, mybir

    mm_dt = getattr(mybir.dt, MM_DT)
    bf16 = mybir.dt.bfloat16

    nc = bacc.Bacc(None)
    # x-shard transpose and refs transpose concatenated along the free dim so
    # ONE DMA loads both matmul operands.
    inT_aug = nc.declare_dram_parameter(
        "inT_aug", [KP, B_SHARD + R], mm_dt, isOutput=False
    )
    out = nc.declare_dram_parameter("out", [B_SHARD, R], bf16, isOutput=True)

    n_bt = B_SHARD // BT
    n_rc = R // RC

    with tile.TileContext(nc) as tc, ExitStack() as ctx:
        consts = ctx.enter_context(tc.tile_pool(name="consts", bufs=1))
        outs = ctx.enter_context(tc.tile_pool(name="outs", bufs=4))
        psums = ctx.enter_context(tc.tile_pool(name="psums", bufs=2, space="PSUM"))

        # Input load, chunked for load/compute overlap and ordered by first
        # need, so the bt=0 matmuls can start ~4us before the full 2 MiB lands.
        x0 = consts.tile([KP, 256], mm_dt)
        r0 = consts.tile([KP, 512], mm_dt)
        r1 = consts.tile([KP, 512], mm_dt)
        r2 = consts.tile([KP, 512], mm_dt)
        r3 = consts.tile([KP, 512], mm_dt)
        x1 = consts.tile([KP, 768], mm_dt)
        x2 = consts.tile([KP, 1024], mm_dt)
        # The PE stream (the steady-state pacer) starts as soon as x0 plus the
        # FIRST 512-col refs chunk land, and the remaining refs chunks arrive
        # just ahead of the matmul that needs them: refs stream in rc-sized
        # [128, 512] chunks on the sync HWDGE ring while the tiny x0 rides the
        # scalar HWDGE ring concurrently. The rest of x queues BEHIND refs on
        # sync so it cannot steal bandwidth from the critical path.
        nc.scalar.dma_start(out=x0, in_=inT_aug[:, 0:256])
        refs_sb = [r0, r1, r2, r3]
        # r0 arrives as two half-DMAs: with subtile dependency tracking, the
        # very first (half-width) matmul fires on the first 128 KB of refs.
        nc.sync.dma_start(out=r0[:, 0:256], in_=inT_aug[:, B_SHARD : B_SHARD + 256])
        nc.sync.dma_start(
            out=r0[:, 256:512], in_=inT_aug[:, B_SHARD + 256 : B_SHARD + 512]
        )
        for i, rt in enumerate(refs_sb[1:], start=1):
            nc.sync.dma_start(
                out=rt, in_=inT_aug[:, B_SHARD + i * RC : B_SHARD + (i + 1) * RC]
            )
        nc.sync.dma_start(out=x1, in_=inT_aug[:, 256:1024])
        nc.sync.dma_start(out=x2, in_=inT_aug[:, 1024:2048])
        refs_off = [0, 0, 0, 0]

        def x_slice(bt):
            if bt < 2:
                return x0, bt * BT
            if bt < 8:
                return x1, (bt - 2) * BT
            return x2, (bt - 8) * BT

        h = R // 2

        # --- tiles 0..14: paired stores (one 1 MiB DMA per two tiles) to cut
        # sync-queue instruction count and dependency-event count (the event
        # drain at kernel end costs ~90ns per event across all engines);
        # bt14 stores alone.
        pair_sb = None
        for bt in range(0, n_bt - 1):
            x_sb, x_off = x_slice(bt)
            ps = psums.tile([BT, R], mybir.dt.float32, tag="ps")
            for rc in range(n_rc):
                if bt == 0 and rc == 0:
                    # Half-width pair so the PE stream starts on r0's first
                    # half-DMA.
                    for c0, c1 in ((0, 256), (256, 512)):
                        nc.tensor.matmul(
                            ps[:, c0:c1],
                            lhsT=x_sb[0:K, x_off : x_off + BT],
                            rhs=refs_sb[0][0:K, c0:c1],
                            start=True,
                            stop=True,
                        )
                    continue
                nc.tensor.matmul(
                    ps[:, rc * RC : (rc + 1) * RC],
                    lhsT=x_sb[0:K, x_off : x_off + BT],
                    rhs=refs_sb[rc][0:K, refs_off[rc] : refs_off[rc] + RC],
                    start=True,
                    stop=True,
                )
            if bt == n_bt - 2:
                sngl_sb = outs.tile([BT, R], bf16, tag="out_sb")
                nc.scalar.activation(
                    sngl_sb,
                    ps,
                    mybir.ActivationFunctionType.Exp,
                    bias=0.0,
                    scale=2.0,
                )
                nc.sync.dma_start(
                    out=out[bt * BT : (bt + 1) * BT, :], in_=sngl_sb
                )
                continue
            if bt % 2 == 0:
                pair_sb = outs.tile([BT, 2 * R], bf16, tag="pair_sb")
            half = bt % 2
            nc.scalar.activation(
                pair_sb[:, half * R : (half + 1) * R],
                ps,
                mybir.ActivationFunctionType.Exp,
                bias=0.0,
                scale=2.0,
            )
            if bt % 2 == 1:
                # One 1 MiB store for the two tiles: DRAM side rearranged so
                # partition p's line maps to rows {bt-1, bt}*BT + p.
                nc.sync.dma_start(
                    out=out[(bt - 1) * BT : (bt + 1) * BT, :].rearrange(
                        "(b p) r -> p b r", b=2
                    ),
                    in_=pair_sb,
                )

        # --- last tile: serial tail ends with a half-size store issued from
        # the scalar queue itself (same-engine program order, no sem hop).
        bt = n_bt - 1
        x_sb, x_off = x_slice(bt)
        ps = psums.tile([BT, R], mybir.dt.float32, tag="ps")
        for rc in range(n_rc):
            nc.tensor.matmul(
                ps[:, rc * RC : (rc + 1) * RC],
                lhsT=x_sb[0:K, x_off : x_off + BT],
                rhs=refs_sb[rc][0:K, refs_off[rc] : refs_off[rc] + RC],
                start=True,
                stop=True,
            )
        out_sb = outs.tile([BT, R], bf16, tag="out_sb")
        q = 3 * R // 4
        for lo, hi, eng in ((0, q, nc.sync), (q, R, nc.scalar)):
            nc.scalar.activation(
                out_sb[:, lo:hi],
                ps[:, lo:hi],
                mybir.ActivationFunctionType.Exp,
                bias=0.0,
                scale=2.0,
            )
            eng.dma_start(
                out=out[bt * BT : (bt + 1) * BT, lo:hi],
                in_=out_sb[:, lo:hi],
            )

    nc.compile()
    return nc


def _hi_lo(v):
    """Split fp64 vector into bf16-representable hi + fp32 remainder lo."""
    import ml_dtypes

    hi = v.astype(np.float32).astype(ml_dtypes.bfloat16).astype(np.float32)
    lo = (v - hi).astype(np.float32)
    return hi, lo


def make_in_maps(x, refs):
    """Host-side prep: shard/transpose x, pack norm terms as extra K rows."""
    x = np.ascontiguousarray(x, dtype=np.float32)
    refs = np.ascontiguousarray(refs, dtype=np.float32)

    r_hi, r_lo = _hi_lo(0.5 * (refs.astype(np.float64) ** 2).sum(axis=1))
    x_sq = 0.5 * (x.astype(np.float64) ** 2).sum(axis=1)  # [B]

    in_maps = []
    for c in range(N_CORES):
        sl = slice(c * B_SHARD, (c + 1) * B_SHARD)
        x_hi, x_lo = _hi_lo(x_sq[sl])
        inT_aug = np.zeros((KP, B_SHARD + R), np.float32)
        inT_aug[:D, :B_SHARD] = x[sl].T
        inT_aug[D, :B_SHARD] = 1.0
        inT_aug[D + 1, :B_SHARD] = 1.0
        inT_aug[D + 2, :B_SHARD] = -x_hi
        inT_aug[D + 3, :B_SHARD] = -x_lo
        inT_aug[:D, B_SHARD:] = refs.T
        inT_aug[D, B_SHARD:] = -r_hi
        inT_aug[D + 1, B_SHARD:] = -r_lo
        inT_aug[D + 2, B_SHARD:] = 1.0
        inT_aug[D + 3, B_SHARD:] = 1.0
        in_maps.append({"inT_aug": inT_aug})
    return in_maps


_NC_CACHE = None


def get_nc():
    global _NC_CACHE
    if _NC_CACHE is None:
        _NC_CACHE = _build_nc()
    return _NC_CACHE


def kernel(x, refs):
    from concourse.bass_utils import run_bass_kernel_spmd

    in_maps = make_in_maps(x, refs)
    res = run_bass_kernel_spmd(
        get_nc(), in_maps, core_ids=list(range(N_CORES))
    ).results
    return np.concatenate(
        [res[c]["out"].astype(np.float32) for c in range(N_CORES)], axis=0
    )


# revision 21
# speedup vs baseline: 1.0840x; 1.0002x over previous
"""RBF kernel feature map: out[b, r] = exp(-||x[b] - refs[r]||^2).

Computed via the GEMM expansion on 8 NeuronCores, data-parallel over the
batch dim of x (2048 rows per core), refs replicated.

Per-core device kernel, per [128, 2048] output tile:
    psum[b, r] = sum_d x[b,d]*refs[r,d] - 0.5*r_sq[r] - 0.5*x_sq[b]
      (4 matmuls of N=512 side by side into one 4-bank PSUM tile)
    out[b, r]  = exp(2 * psum[b, r])   (ONE ScalarE Exp over N=2048)

The contraction packs all the norm terms into 4 extra K rows (vs ones-rows
on the opposite side). Each norm term is split hi (bf16-representable) +
lo (remainder) so the full-rate fp32r matmul path keeps the ~30..300
magnitude norm terms accurate even if fp32r quantizes inputs to ~tf32
internally.

Perf notes vs the v1 kernel (79.6us -> ~54.5us measured):
  - input DRAM param padded from [68, B+R] to [128, B+R]: a 68-partition
    DMA was served by only 4 SDMA engines (77 GB/s, 14.4us startup
    stall); 128 partitions spread over all 16 engines (~330 GB/s). The
    60 zero rows cost bytes but not time (engine count ~ partitions/8).
  - output stored as bf16 and upcast on the host: halves the 16 MiB/core
    store traffic that ran at the ~345 GB/s per-core HBM ceiling.
    exp() outputs are in [0, ~1e-13]; bf16 adds <=2e-3 relative error
    against a 2e-2 budget (measured baseline error 2.8e-3).
  - one Exp ACTIVATE per [128, 2048] PSUM region instead of four per
    [128, 512]: the ~170-350 cycle per-instruction bubble made 64 small
    ACTs cost 59.7us of ScalarE time; 16 big ones cost ~31.4us.
  - input loaded in need-ordered chunks: x cols 0:256 on the scalar
    HWDGE ring, refs as four rc-sized [128, 512] chunks first on the
    sync ring (rest of x behind them). The PE stream - the steady-state
    pacer - starts after just x0+r0 (~384 KB) instead of the full 2 MiB,
    and each bt=0 matmul's refs chunk lands just ahead of its use.
  - middle tiles store in 1 MiB pairs (DRAM AP rearranged "(b p) r ->
    p b r") to cut sync-queue instructions and dependency events; the
    last tile ends with a half-size store issued from the scalar queue.
  - steady state is PE-paced at ~520ns per 512-col fp32r matmul (the PE
    stays at 1.2 GHz here - HAM warm-up via dummy matmuls was tried and
    does not engage; measured identical 628ns for bf16 and fp32r), with
    the 1.96us Exp per tile just underneath. ~7us NEFF prologue and
    ~3us event-drain epilogue are runtime-fixed.

Uses bacc.Bacc (not raw bass.Bass): TRN2 instructions carry at most one
semaphore wait, and Bacc.compile()'s generate_event_semaphores pass
legalizes the multi-wait instructions Tile emits.
"""

import numpy as np

N_CORES = 8
B, D, R = 16384, 64, 2048
B_SHARD = B // N_CORES  # 2048
K = D + 4  # 64 data rows + r_sq hi/lo + x_sq hi/lo rows
KP = 128  # padded partition count so the input DMA uses all 16 SDMA engines
BT = 128  # batch rows per tile (PSUM partition dim)
RC = 512  # refs cols per matmul (max fp32 moving free dim)

MM_DT = "float32r"  # full-rate fp32 matmul mode; "float32" = 4x slower, exact


def _build_nc():
    from contextlib import ExitStack

    import concourse.tile as tile
    from concourse import bacc, mybir

    mm_dt = getattr(mybir.dt, MM_DT)
    bf16 = mybir.dt.bfloat16

    nc = bacc.Bacc(None)
    # x-shard transpose and refs transpose concatenated along the free dim so
    # ONE DMA loads both matmul operands.
    inT_aug = nc.declare_dram_parameter(
        "inT_aug", [KP, B_SHARD + R], mm_dt, isOutput=False
    )
    out = nc.declare_dram_parameter("out", [B_SHARD, R], bf16, isOutput=True)

    n_bt = B_SHARD // BT
    n_rc = R // RC

    with tile.TileContext(nc) as tc, ExitStack() as ctx:
        consts = ctx.enter_context(tc.tile_pool(name="consts", bufs=1))
        outs = ctx.enter_context(tc.tile_pool(name="outs", bufs=4))
        psums = ctx.enter_context(tc.tile_pool(name="psums", bufs=2, space="PSUM"))

        # Input load, chunked for load/compute overlap and ordered by first
        # need, so the bt=0 matmuls can start ~4us before the full 2 MiB lands.
        x0 = consts.tile([KP, 256], mm_dt)
        r0 = consts.tile([KP, 512], mm_dt)
        r1 = consts.tile([KP, 512], mm_dt)
        r2 = consts.tile([KP, 512], mm_dt)
        r3 = consts.tile([KP, 512], mm_dt)
        x1 = consts.tile([KP, 768], mm_dt)
        x2 = consts.tile([KP, 1024], mm_dt)
        # The PE stream (the steady-state pacer) starts as soon as x0 plus the
        # FIRST 512-col refs chunk land, and the remaining refs chunks arrive
        # just ahead of the matmul that needs them: refs stream in rc-sized
        # [128, 512] chunks on the sync HWDGE ring while the tiny x0 rides the
        # scalar HWDGE ring concurrently. The rest of x queues BEHIND refs on
        # sync so it cannot steal bandwidth from the critical path.
        nc.scalar.dma_start(out=x0, in_=inT_aug[:, 0:256])
        refs_sb = [r0, r1, r2, r3]
        # r0 arrives as two half-DMAs: with subtile dependency tracking, the
        # very first (half-width) matmul fires on the first 128 KB of refs.
        nc.sync.dma_start(out=r0[:, 0:256], in_=inT_aug[:, B_SHARD : B_SHARD + 256])
        nc.sync.dma_start(
            out=r0[:, 256:512], in_=inT_aug[:, B_SHARD + 256 : B_SHARD + 512]
        )
        for i, rt in enumerate(refs_sb[1:], start=1):
            nc.sync.dma_start(
                out=rt, in_=inT_aug[:, B_SHARD + i * RC : B_SHARD + (i + 1) * RC]
            )
        nc.sync.dma_start(out=x1, in_=inT_aug[:, 256:1024])
        nc.sync.dma_start(out=x2, in_=inT_aug[:, 1024:2048])
        refs_off = [0, 0, 0, 0]

        def x_slice(bt):
            if bt < 2:
                return x0, bt * BT
            if bt < 8:
                return x1, (bt - 2) * BT
            return x2, (bt - 8) * BT

        h = R // 2

        # --- tiles 0..14: paired stores (one 1 MiB DMA per two tiles) to cut
        # sync-queue instruction count and dependency-event count (the event
        # drain at kernel end costs ~90ns per event across all engines);
        # bt14 stores alone.
        pair_sb = None
        for bt in range(0, n_bt - 1):
            x_sb, x_off = x_slice(bt)
            ps = psums.tile([BT, R], mybir.dt.float32, tag="ps")
            for rc in range(n_rc):
                if bt == 0 and rc == 0:
                    # Half-width pair so the PE stream starts on r0's first
                    # half-DMA.
                    for c0, c1 in ((0, 256), (256, 512)):
                        nc.tensor.matmul(
                            ps[:, c0:c1],
                            lhsT=x_sb[0:K, x_off : x_off + BT],
                            rhs=refs_sb[0][0:K, c0:c1],
                            start=True,
                            stop=True,
                        )
                    continue
                nc.tensor.matmul(
                    ps[:, rc * RC : (rc + 1) * RC],
                    lhsT=x_sb[0:K, x_off : x_off + BT],
                    rhs=refs_sb[rc][0:K, refs_off[rc] : refs_off[rc] + RC],
                    start=True,
                    stop=True,
                )
            if bt == n_bt - 2:
                sngl_sb = outs.tile([BT, R], bf16, tag="out_sb")
                nc.scalar.activation(
                    sngl_sb,
                    ps,
                    mybir.ActivationFunctionType.Exp,
                    bias=0.0,
                    scale=2.0,
                )
                nc.sync.dma_start(
                    out=out[bt * BT : (bt + 1) * BT, :], in_=sngl_sb
                )
                continue
            if bt % 2 == 0:
                pair_sb = outs.tile([BT, 2 * R], bf16, tag="pair_sb")
            half = bt % 2
            nc.scalar.activation(
                pair_sb[:, half * R : (half + 1) * R],
                ps,
                mybir.ActivationFunctionType.Exp,
                bias=0.0,
                scale=2.0,
            )
            if bt % 2 == 1:
                # One 1 MiB store for the two tiles: DRAM side rearranged so
                # partition p's line maps to rows {bt-1, bt}*BT + p.
                nc.sync.dma_start(
                    out=out[(bt - 1) * BT : (bt + 1) * BT, :].rearrange(
                        "(b p) r -> p b r", b=2
                    ),
                    in_=pair_sb,
                )

        # --- last tile: serial tail ends with a half-size store issued from
        # the scalar queue itself (same-engine program order, no sem hop).
        bt = n_bt - 1
        x_sb, x_off = x_slice(bt)
        q = 3 * RC  # 1536
        # Last tile in TWO psum slots (3 MMs + 1 MM): the 3/4-width Exp fires
        # while the fourth matmul still streams, so only the 1/4-width Exp and
        # a 128 KB scalar-issued store remain serial after the PE finishes.
        psA = psums.tile([BT, R], mybir.dt.float32, tag="ps")
        psB = psums.tile([BT, R], mybir.dt.float32, tag="ps")
        for rc in range(3):
            nc.tensor.matmul(
                psA[:, rc * RC : (rc + 1) * RC],
                lhsT=x_sb[0:K, x_off : x_off + BT],
                rhs=refs_sb[rc][0:K, refs_off[rc] : refs_off[rc] + RC],
                start=True,
                stop=True,
            )
        nc.tensor.matmul(
            psB[:, 0:RC],
            lhsT=x_sb[0:K, x_off : x_off + BT],
            rhs=refs_sb[3][0:K, refs_off[3] : refs_off[3] + RC],
            start=True,
            stop=True,
        )
        out_sb = outs.tile([BT, R], bf16, tag="out_sb")
        nc.scalar.activation(
            out_sb[:, 0:q],
            psA[:, 0:q],
            mybir.ActivationFunctionType.Exp,
            bias=0.0,
            scale=2.0,
        )
        nc.sync.dma_start(
            out=out[bt * BT : (bt + 1) * BT, 0:q], in_=out_sb[:, 0:q]
        )
        nc.scalar.activation(
            out_sb[:, q:R],
            psB[:, 0:RC],
            mybir.ActivationFunctionType.Exp,
            bias=0.0,
            scale=2.0,
        )
        nc.scalar.dma_start(
            out=out[bt * BT : (bt + 1) * BT, q:R], in_=out_sb[:, q:R]
        )

    nc.compile()
    return nc


def _hi_lo(v):
    """Split fp64 vector into bf16-representable hi + fp32 remainder lo."""
    import ml_dtypes

    hi = v.astype(np.float32).astype(ml_dtypes.bfloat16).astype(np.float32)
    lo = (v - hi).astype(np.float32)
    return hi, lo


def make_in_maps(x, refs):
    """Host-side prep: shard/transpose x, pack norm terms as extra K rows."""
    x = np.ascontiguousarray(x, dtype=np.float32)
    refs = np.ascontiguousarray(refs, dtype=np.float32)

    r_hi, r_lo = _hi_lo(0.5 * (refs.astype(np.float64) ** 2).sum(axis=1))
    x_sq = 0.5 * (x.astype(np.float64) ** 2).sum(axis=1)  # [B]

    in_maps = []
    for c in range(N_CORES):
        sl = slice(c * B_SHARD, (c + 1) * B_SHARD)
        x_hi, x_lo = _hi_lo(x_sq[sl])
        inT_aug = np.zeros((KP, B_SHARD + R), np.float32)
        inT_aug[:D, :B_SHARD] = x[sl].T
        inT_aug[D, :B_SHARD] = 1.0
        inT_aug[D + 1, :B_SHARD] = 1.0
        inT_aug[D + 2, :B_SHARD] = -x_hi
        inT_aug[D + 3, :B_SHARD] = -x_lo
        inT_aug[:D, B_SHARD:] = refs.T
        inT_aug[D, B_SHARD:] = -r_hi
        inT_aug[D + 1, B_SHARD:] = -r_lo
        inT_aug[D + 2, B_SHARD:] = 1.0
        inT_aug[D + 3, B_SHARD:] = 1.0
        in_maps.append({"inT_aug": inT_aug})
    return in_maps


_NC_CACHE = None


def get_nc():
    global _NC_CACHE
    if _NC_CACHE is None:
        _NC_CACHE = _build_nc()
    return _NC_CACHE


def kernel(x, refs):
    from concourse.bass_utils import run_bass_kernel_spmd

    in_maps = make_in_maps(x, refs)
    res = run_bass_kernel_spmd(
        get_nc(), in_maps, core_ids=list(range(N_CORES))
    ).results
    return np.concatenate(
        [res[c]["out"].astype(np.float32) for c in range(N_CORES)], axis=0
    )
